# revision 1
# baseline (speedup 1.0000x reference)
"""Cross-attention with structure (relative-position) bias, distributed
across 8 Trainium2 NeuronCores.

Sharding: sequence-parallel over query positions (L=1024 -> 8 chunks of
128). Each core gets its query chunk for all batches/heads plus the
matching structure_matrix slice S[l_chunk, :, :] (so the 268 MB structure
tensor is read exactly once across the fleet, 33.5 MB/core).  K/V
projections are computed per-core from the full hidden_states_value
(replicated compute, no collectives needed).  Softmax rows are complete
on-core (full key axis), so outputs are pure concatenations over l —
no all-reduce required.
"""

import math

import numpy as np

B, L, HID, H, DH = 4, 1024, 1024, 16, 64
EPS = 1e-5
N_CORES = 8
LC = L // N_CORES  # 128 query positions per core


def _per_core(hsq_c, hsv, mask, S_c, WqT, bq, WkT, bk, WvT, bv, WdT, bd, g, beta):
    """One core's shard: hsq_c [B,LC,HID], S_c [LC,L,DH]; returns
    (out_c [B,LC,HID], scores_c [B,H,LC,L]).  Written in jax.numpy when
    traced under pmap, plain numpy otherwise (jnp/np API-compatible)."""
    import jax.numpy as jnp

    q = (hsq_c @ WqT + bq).reshape(B, LC, H, DH).transpose(0, 2, 1, 3)
    k = (hsv @ WkT + bk).reshape(B, L, H, DH).transpose(0, 2, 1, 3)
    v = (hsv @ WvT + bv).reshape(B, L, H, DH).transpose(0, 2, 1, 3)

    scores = jnp.einsum('bhld,bhrd->bhlr', q, k)
    scores = scores + jnp.einsum('bhld,lrd->bhlr', q, S_c)
    scores = scores + jnp.einsum('bhrd,lrd->bhlr', k, S_c)
    scores = scores / math.sqrt(DH)
    scores = scores + mask  # [B,1,1,L] broadcast

    m = jnp.max(scores, axis=-1, keepdims=True)
    e = jnp.exp(scores - m)
    probs = e / jnp.sum(e, axis=-1, keepdims=True)

    ctx = jnp.einsum('bhlr,bhrd->bhld', probs, v)
    ctx = ctx.transpose(0, 2, 1, 3).reshape(B, LC, H * DH)

    out = ctx @ WdT + bd + hsq_c
    mu = jnp.mean(out, axis=-1, keepdims=True)
    var = jnp.mean(jnp.square(out - mu), axis=-1, keepdims=True)
    out = (out - mu) / jnp.sqrt(var + EPS) * g + beta
    return out, scores


def _run_neuron(inputs):
    import jax

    devs = jax.devices()
    if len(devs) < N_CORES:
        raise RuntimeError(f"need {N_CORES} cores, have {len(devs)}")

    hsq = inputs["hidden_states_query"]
    hsv = inputs["hidden_states_value"]
    mask = inputs["attention_mask"]
    S = inputs["structure_matrix"]

    f32 = np.float32
    WqT = np.ascontiguousarray(inputs["Wq"].T, f32)
    WkT = np.ascontiguousarray(inputs["Wk"].T, f32)
    WvT = np.ascontiguousarray(inputs["Wv"].T, f32)
    WdT = np.ascontiguousarray(inputs["Wd"].T, f32)

    # stack per-core shards on a leading axis for pmap
    hsq_s = np.stack([hsq[:, c * LC:(c + 1) * LC, :] for c in range(N_CORES)])
    S_s = np.stack([S[c * LC:(c + 1) * LC] for c in range(N_CORES)])

    def rep(x):
        return np.broadcast_to(x, (N_CORES,) + x.shape)

    fn = jax.pmap(_per_core, devices=devs[:N_CORES])
    out_s, scores_s = fn(
        hsq_s, rep(hsv), rep(mask), S_s,
        rep(WqT), rep(inputs["bq"]), rep(WkT), rep(inputs["bk"]),
        rep(WvT), rep(inputs["bv"]), rep(WdT), rep(inputs["bd"]),
        rep(inputs["ln_gamma"]), rep(inputs["ln_beta"]),
    )
    out_s = np.asarray(out_s)      # [8, B, LC, HID]
    scores_s = np.asarray(scores_s)  # [8, B, H, LC, L]

    out = np.concatenate([out_s[c] for c in range(N_CORES)], axis=1)
    scores = np.concatenate([scores_s[c] for c in range(N_CORES)], axis=2)
    return out.astype(f32), scores.astype(f32)


def _run_cpu(inputs):
    f32 = np.float32
    hsq = np.asarray(inputs["hidden_states_query"], f32)
    hsv = np.asarray(inputs["hidden_states_value"], f32)
    mask = np.asarray(inputs["attention_mask"], f32)
    S = np.asarray(inputs["structure_matrix"], f32)
    WqT = np.ascontiguousarray(np.asarray(inputs["Wq"], f32).T)
    WkT = np.ascontiguousarray(np.asarray(inputs["Wk"], f32).T)
    WvT = np.ascontiguousarray(np.asarray(inputs["Wv"], f32).T)
    WdT = np.ascontiguousarray(np.asarray(inputs["Wd"], f32).T)
    bq, bk, bv, bd = (np.asarray(inputs[n], f32) for n in ("bq", "bk", "bv", "bd"))
    g, beta = np.asarray(inputs["ln_gamma"], f32), np.asarray(inputs["ln_beta"], f32)

    q = (hsq @ WqT + bq).reshape(B, L, H, DH).transpose(0, 2, 1, 3)
    k = (hsv @ WkT + bk).reshape(B, L, H, DH).transpose(0, 2, 1, 3)
    v = (hsv @ WvT + bv).reshape(B, L, H, DH).transpose(0, 2, 1, 3)

    scores = np.einsum('bhld,bhrd->bhlr', q, k, optimize=True)
    # per-l BLAS matmuls: q_l [B*H, DH] @ S[l].T [DH, L]
    qf = np.ascontiguousarray(q.transpose(2, 0, 1, 3).reshape(L, B * H, DH))
    for l in range(L):
        scores[:, :, l, :] += (qf[l] @ S[l].T).reshape(B, H, L)
    # k-term: for each l, rowwise dot of k[:,:,r,:] with S[l,r,:]
    kf = np.ascontiguousarray(k.transpose(0, 1, 3, 2))  # [B,H,DH,L]
    for l in range(L):
        # S[l] is [L, DH]; sum_d k[b,h,r,d]*S[l,r,d]
        scores[:, :, l, :] += np.einsum('bhdr,rd->bhr', kf, S[l], optimize=True)
    scores /= math.sqrt(DH)
    scores += mask

    m = scores.max(axis=-1, keepdims=True)
    e = np.exp(scores - m)
    probs = e / e.sum(axis=-1, keepdims=True)
    ctx = np.einsum('bhlr,bhrd->bhld', probs, v, optimize=True)
    ctx = ctx.transpose(0, 2, 1, 3).reshape(B, L, H * DH)

    out = ctx @ WdT + bd + hsq
    mu = out.mean(axis=-1, keepdims=True)
    var = ((out - mu) ** 2).mean(axis=-1, keepdims=True)
    out = (out - mu) / np.sqrt(var + EPS) * g + beta
    return out.astype(f32), scores.astype(f32)


def kernel(**inputs):
    inputs = {k: np.asarray(v) for k, v in inputs.items()}
    try:
        return _run_neuron(inputs)
    except Exception as e:  # no devices / compile failure -> CPU fallback
        import sys
        print(f"kernel: neuron path failed ({type(e).__name__}: {e}); "
              f"falling back to CPU", file=sys.stderr)
        return _run_cpu(inputs)
